# revision 14
# baseline (speedup 1.0000x reference)
"""Trainium2 Bass kernel for CrossframeGlobalAttentionModule.

Reference computation (N=500000 current vertices, N_PREV=450000 previous,
C=64 channels, G=32 groups):
    h  = h_lv @ W_hidden.T + b_hidden            # [N_PREV, C]
    h  = pad(h, N)                               # zero rows N_PREV..N
    h  = relu(h @ W_conv.T)
    h  = group_norm(h, gamma, beta)              # stats over ALL N rows
    g  = sigmoid((h @ W_conv.T) / (N + C))
    g[N_PREV:] = 1.0
    out = g * lv

Key observations exploited here:
  * Rows >= N_PREV of the padded h are zero, relu keeps them zero, so they
    contribute nothing to the group sums; their gate is overwritten with 1.0.
    Only the first N_PREV rows need the full pipeline; rows N_PREV..N are a
    pure copy of lv.  The zero rows still count toward the group-norm divisor
    (N * C/G elements per group), which is handled analytically.
  * The whole h-pipeline can run in bf16: the pre-sigmoid value is
    ~1e-5 in magnitude, so gate = sigmoid(z) = 0.5 + z/4 + ...; a 0.4%
    relative error in z perturbs the output by ~1e-8 relative.  lv and the
    final gate*lv multiply stay fp32.
  * The group-norm affine (x - mean) * gamma * rstd + beta followed by the
    second W_conv matmul is folded into the matmul:
        Wc @ (s*h + t) = (Wc * s) @ h + Wc @ t
    so phase 2 is a single matmul with runtime-scaled weights plus a
    per-channel bias that rides on the sigmoid activation's bias input.

Distribution: data-parallel over the vertex dim on 8 cores.  Each core gets
56250 rows of h_lv/lv (plus 6250 passthrough rows), stored transposed
([C, rows], packed host-side) so channels sit on SBUF partitions.  Two
28125-row blocks are packed into the 128 partitions and processed with
block-diagonal 128x128 weights, giving full PE/DVE/ACT width.  Group-norm
statistics need one 256-byte AllReduce of per-group sum/sumsq.
"""

import math

import numpy as np
import ml_dtypes

import concourse.bass as bass
import concourse.tile as tile
from concourse import bacc, mybir
from concourse.bass_utils import run_bass_kernel_spmd

# ---- problem constants (hardcoded; kernel.py must be self-contained) ----
N_FULL = 500000
N_PREV = 450000
C = 64
G = 32
EPS = 1e-5
NCORES = 8

RH = N_PREV // NCORES            # 56250 gate rows per core
RP = (N_FULL - N_PREV) // NCORES  # 6250 passthrough rows per core
HALF = RH // 2                   # 28125 packed columns (2 blocks of rows)
CSCALE = 1.0 / (N_FULL + C)
INV_CNT = 1.0 / (N_FULL * (C // G))  # group-norm divisor (zeros included)

FD = 2048    # DMA / DVE chunk width (columns)
FDA = 1024   # ACT / PSUM chunk width
MM = 512     # single-matmul moving-operand width (one PSUM bank, fp32 out)

F32 = mybir.dt.float32
BF16 = mybir.dt.bfloat16
AX = mybir.AxisListType
ALU = mybir.AluOpType
ACTF = mybir.ActivationFunctionType

NT = math.ceil(HALF / FD)    # 14 outer chunks
NA = math.ceil(HALF / FDA)   # 28 ACT chunks


def _ceil_chunks(total, step):
    return [(i, min(step, total - i)) for i in range(0, total, step)]


def build_nc(ncores=NCORES, use_collective=True):
    nc = bacc.Bacc(
        "TRN2", target_bir_lowering=False, debug=False, num_devices=ncores
    )

    hT = nc.dram_tensor("hT", [C, RH], BF16, kind="ExternalInput").ap()
    lvT = nc.dram_tensor("lvT", [C, RH], F32, kind="ExternalInput").ap()
    lvTp = nc.dram_tensor("lvTp", [C, RP], F32, kind="ExternalInput").ap()
    whT_d = nc.dram_tensor("whT", [128, 128], BF16, kind="ExternalInput").ap()
    wcT_d = nc.dram_tensor("wcT", [128, 128], BF16, kind="ExternalInput").ap()
    biash_d = nc.dram_tensor("biash", [128, 1], F32, kind="ExternalInput").ap()
    gam_d = nc.dram_tensor("gam", [128, 1], F32, kind="ExternalInput").ap()
    bet_d = nc.dram_tensor("bet", [128, 1], F32, kind="ExternalInput").ap()
    selA_d = nc.dram_tensor("selA", [128, G], F32, kind="ExternalInput").ap()
    selB_d = nc.dram_tensor("selB", [G, 128], F32, kind="ExternalInput").ap()
    outT = nc.dram_tensor("outT", [C, RH + RP], F32, kind="ExternalOutput").ap()

    # [C, 2*HALF] -> [2, C, HALF]: DMA'd against [128, L] SBUF tiles so
    # partition (b*64+c) holds channel c of row-block b (same linearization).
    hTv = hT.rearrange("c (b l) -> b c l", b=2)
    lvTv = lvT.rearrange("c (b l) -> b c l", b=2)
    outTv = outT[:, 0:RH].rearrange("c (b l) -> b c l", b=2)

    with tile.TileContext(nc) as tc:
        with (
            tc.tile_pool(name="const", bufs=1) as constp,
            tc.tile_pool(name="load", bufs=3) as loadp,
            tc.tile_pool(name="h1p", bufs=2) as h1p,
            tc.tile_pool(name="h2p", bufs=1) as h2p,
            tc.tile_pool(name="gatep", bufs=2) as gatep,
            tc.tile_pool(name="outp", bufs=3) as outp,
            tc.tile_pool(name="statp", bufs=1) as statp,
            tc.tile_pool(name="psA", bufs=2, space="PSUM") as psA,
            tc.tile_pool(name="psB", bufs=2, space="PSUM") as psB,
            tc.tile_pool(name="dram", bufs=1, space="DRAM") as dramp,
        ):
            # ---- constants to SBUF ----
            whT = constp.tile([128, 128], BF16, tag="whT")
            wcT = constp.tile([128, 128], BF16, tag="wcT")
            biash = constp.tile([128, 1], F32, tag="biash")
            gam = constp.tile([128, 1], F32, tag="gam")
            bet = constp.tile([128, 1], F32, tag="bet")
            selA = constp.tile([128, G], F32, tag="selA")
            selB = constp.tile([G, 128], F32, tag="selB")
            nc.sync.dma_start(whT[:], whT_d)
            nc.sync.dma_start(wcT[:], wcT_d)
            nc.sync.dma_start(biash[:], biash_d)
            nc.sync.dma_start(gam[:], gam_d)
            nc.sync.dma_start(bet[:], bet_d)
            nc.sync.dma_start(selA[:], selA_d)
            nc.sync.dma_start(selB[:], selB_d)

            # Dummy Sqrt up front so the ACT sqrt table set loads during the
            # initial DMAs; relu/identity/copy live in every set, so the only
            # mid-kernel table switch left is the post-stats sigmoid load.
            warm = statp.tile([128, 1], F32, tag="warm")
            nc.vector.memset(warm[:], 1.0)
            warm2 = statp.tile([128, 1], F32, tag="warm2")
            nc.scalar.activation(warm2[:], warm[:], ACTF.Sqrt)

            # passthrough rows: single DRAM->DRAM copy (gate == 1.0 there)
            nc.sync.dma_start(outT[:, RH : RH + RP], lvTp)

            # h2 stays resident in SBUF for phase 2 (bf16, 56.25KB/partition)
            h2 = h2p.tile([128, HALF], BF16, tag="h2")
            # bn_stats (count, mean, M2) x2 per 512-col sub-chunk
            nsub = len(_ceil_chunks(HALF, MM))
            stat6 = statp.tile([128, 6 * nsub], F32, tag="stat6")

            # ---- phase 1: h2 = relu(Wc_bd @ (Wh_bd @ hT + b)) + stats ----
            for j, (c0, lw) in enumerate(_ceil_chunks(HALF, FD)):
                ht = loadp.tile([128, FD], BF16, tag="ht")
                nc.sync.dma_start(ht[:, 0:lw], hTv[:, :, c0 : c0 + lw])
                for k, (a0, la) in enumerate(_ceil_chunks(lw, FDA)):
                    pa = psA.tile([128, FDA], F32, tag="a")
                    for m0, lm in _ceil_chunks(la, MM):
                        nc.tensor.matmul(
                            pa[:, m0 : m0 + lm],
                            whT[:],
                            ht[:, a0 + m0 : a0 + m0 + lm],
                            start=True,
                            stop=True,
                        )
                    h1 = h1p.tile([128, FDA], BF16, tag="h1")
                    nc.scalar.activation(
                        h1[:, 0:la], pa[:, 0:la], ACTF.Identity, bias=biash[:, 0:1]
                    )
                    pb = psB.tile([128, FDA], F32, tag="b")
                    for m0, lm in _ceil_chunks(la, MM):
                        nc.tensor.matmul(
                            pb[:, m0 : m0 + lm],
                            wcT[:],
                            h1[:, m0 : m0 + lm],
                            start=True,
                            stop=True,
                        )
                    # relu into resident h2
                    nc.scalar.activation(
                        h2[:, c0 + a0 : c0 + a0 + la],
                        pb[:, 0:la],
                        ACTF.Relu,
                    )
                    # per-512 running stats of the freshly-written h2
                    for s0, ls in _ceil_chunks(la, MM):
                        si = (c0 + a0 + s0) // MM
                        nc.vector.bn_stats(
                            stat6[:, 6 * si : 6 * si + 6],
                            h2[:, c0 + a0 + s0 : c0 + a0 + s0 + ls],
                        )

            # ---- stats: per-partition -> per-group -> AllReduce -> affine ----
            agg = statp.tile([128, 2], F32, tag="agg")
            nc.vector.bn_aggr(agg[:], stat6[:])
            # convert (mean, var) over HALF cols -> (sum, sumsq)
            msq0 = statp.tile([128, 1], F32, tag="msq0")
            nc.vector.tensor_tensor(msq0[:], agg[:, 0:1], agg[:, 0:1], ALU.mult)
            ex20 = statp.tile([128, 1], F32, tag="ex20")
            nc.vector.tensor_tensor(ex20[:], agg[:, 1:2], msq0[:], ALU.add)
            ssum = statp.tile([128, 2], F32, tag="ssum")
            nc.vector.tensor_scalar_mul(ssum[:, 0:1], agg[:, 0:1], float(HALF))
            nc.vector.tensor_scalar_mul(ssum[:, 1:2], ex20[:], float(HALF))
            pg = psA.tile([32, 2], F32, tag="a")
            nc.tensor.matmul(pg[:], selA[:], ssum[:], start=True, stop=True)
            gsb = statp.tile([32, 2], F32, tag="gsb")
            nc.scalar.copy(gsb[:], pg[:])

            cin = dramp.tile([32, 2], F32, tag="cin")
            cout = dramp.tile([32, 2], F32, tag="cout")
            nc.sync.dma_start(cin[:], gsb[:])
            if use_collective:
                nc.gpsimd.collective_compute(
                    "AllReduce",
                    ALU.add,
                    ins=[cin.opt()],
                    outs=[cout.opt()],
                    replica_groups=[list(range(ncores))],
                )
            else:
                nc.sync.dma_start(cout[:], cin[:])
            gall = statp.tile([32, 2], F32, tag="gall")
            nc.sync.dma_start(gall[:], cout[:])

            pbc = psB.tile([128, 2], F32, tag="b")
            nc.tensor.matmul(pbc[:], selB[:], gall[:], start=True, stop=True)
            mean = statp.tile([128, 1], F32, tag="mean")
            ex2 = statp.tile([128, 1], F32, tag="ex2")
            nc.vector.tensor_scalar_mul(mean[:], pbc[:, 0:1], INV_CNT)
            nc.vector.tensor_scalar_mul(ex2[:], pbc[:, 1:2], INV_CNT)
            msq = statp.tile([128, 1], F32, tag="msq")
            nc.vector.tensor_tensor(msq[:], mean[:], mean[:], ALU.mult)
            veps = statp.tile([128, 1], F32, tag="veps")
            nc.vector.tensor_tensor(veps[:], ex2[:], msq[:], ALU.subtract)
            nc.vector.tensor_scalar_add(veps[:], veps[:], EPS)
            urec = statp.tile([128, 1], F32, tag="urec")
            nc.vector.reciprocal(urec[:], veps[:])
            rstd = statp.tile([128, 1], F32, tag="rstd")
            nc.scalar.activation(rstd[:], urec[:], ACTF.Sqrt)

            svec = statp.tile([128, 1], F32, tag="svec")
            nc.vector.tensor_tensor(svec[:], gam[:], rstd[:], ALU.mult)
            mstmp = statp.tile([128, 1], F32, tag="mstmp")
            nc.vector.tensor_tensor(mstmp[:], mean[:], svec[:], ALU.mult)
            tvec = statp.tile([128, 1], F32, tag="tvec")
            nc.vector.tensor_tensor(tvec[:], bet[:], mstmp[:], ALU.subtract)
            tbf = statp.tile([128, 1], BF16, tag="tbf")
            nc.vector.tensor_copy(tbf[:], tvec[:])

            w2 = constp.tile([128, 128], BF16, tag="w2")
            nc.vector.tensor_scalar_mul(w2[:], wcT[:], svec[:, 0:1])
            pbias = psB.tile([128, 1], F32, tag="b")
            nc.tensor.matmul(pbias[:], wcT[:], tbf[:], start=True, stop=True)
            sigb = statp.tile([128, 1], F32, tag="sigb")
            nc.vector.tensor_scalar_mul(sigb[:], pbias[:], CSCALE)

            # ---- phase 2: gate = sigmoid((W2_bd @ h2)*c + sigb); out=gate*lv
            for j, (c0, lw) in enumerate(_ceil_chunks(HALF, FD)):
                lt = loadp.tile([128, FD], F32, tag="lt")
                nc.sync.dma_start(lt[:, 0:lw], lvTv[:, :, c0 : c0 + lw])
                gate = gatep.tile([128, FD], F32, tag="g")
                for a0, la in _ceil_chunks(lw, FDA):
                    pc = psA.tile([128, FDA], F32, tag="a")
                    for m0, lm in _ceil_chunks(la, MM):
                        nc.tensor.matmul(
                            pc[:, m0 : m0 + lm],
                            w2[:],
                            h2[:, c0 + a0 + m0 : c0 + a0 + m0 + lm],
                            start=True,
                            stop=True,
                        )
                    nc.scalar.activation(
                        gate[:, a0 : a0 + la],
                        pc[:, 0:la],
                        ACTF.Sigmoid,
                        bias=sigb[:, 0:1],
                        scale=CSCALE,
                    )
                ot = outp.tile([128, FD], F32, tag="o")
                nc.vector.tensor_tensor(
                    ot[:, 0:lw], gate[:, 0:lw], lt[:, 0:lw], ALU.mult
                )
                nc.sync.dma_start(outTv[:, :, c0 : c0 + lw], ot[:, 0:lw])

    nc.compile()
    return nc


_NC_CACHE = None


def _get_nc():
    global _NC_CACHE
    if _NC_CACHE is None:
        _NC_CACHE = build_nc()
    return _NC_CACHE


def _prep_consts(W_hidden, b_hidden, W_conv, gamma, beta):
    whT = np.zeros((128, 128), np.float32)
    wcT = np.zeros((128, 128), np.float32)
    whT[0:64, 0:64] = W_hidden.T
    whT[64:128, 64:128] = W_hidden.T
    wcT[0:64, 0:64] = W_conv.T
    wcT[64:128, 64:128] = W_conv.T
    biash = np.concatenate([b_hidden, b_hidden]).reshape(128, 1).astype(np.float32)
    gam = np.concatenate([gamma, gamma]).reshape(128, 1).astype(np.float32)
    bet = np.concatenate([beta, beta]).reshape(128, 1).astype(np.float32)
    p = np.arange(128)
    selA = ((p[:, None] % 64) // 2 == np.arange(G)[None, :]).astype(np.float32)
    selB = np.ascontiguousarray(selA.T)
    return {
        "whT": whT.astype(ml_dtypes.bfloat16),
        "wcT": wcT.astype(ml_dtypes.bfloat16),
        "biash": biash,
        "gam": gam,
        "bet": bet,
        "selA": selA,
        "selB": selB,
    }


def kernel(lv, h_lv, W_hidden, b_hidden, W_conv, gamma, beta, _trace=False):
    lv = np.asarray(lv, np.float32)
    h_lv = np.asarray(h_lv, np.float32)
    consts = _prep_consts(
        np.asarray(W_hidden, np.float32),
        np.asarray(b_hidden, np.float32),
        np.asarray(W_conv, np.float32),
        np.asarray(gamma, np.float32),
        np.asarray(beta, np.float32),
    )

    in_maps = []
    for i in range(NCORES):
        hs = h_lv[i * RH : (i + 1) * RH]
        ls = lv[i * RH : (i + 1) * RH]
        ps = lv[N_PREV + i * RP : N_PREV + (i + 1) * RP]
        m = dict(consts)
        m["hT"] = np.ascontiguousarray(hs.T).astype(ml_dtypes.bfloat16)
        m["lvT"] = np.ascontiguousarray(ls.T)
        m["lvTp"] = np.ascontiguousarray(ps.T)
        in_maps.append(m)

    nc = _get_nc()
    res = run_bass_kernel_spmd(
        nc, in_maps, core_ids=list(range(NCORES)), trace=_trace
    )

    out = np.empty((N_FULL, C), np.float32)
    for i in range(NCORES):
        o = res.results[i]["outT"]
        out[i * RH : (i + 1) * RH] = o[:, 0:RH].T
        out[N_PREV + i * RP : N_PREV + (i + 1) * RP] = o[:, RH : RH + RP].T
    if _trace:
        return out, res
    return out


# revision 15
# speedup vs baseline: 3.3253x; 3.3253x over previous
"""Trainium2 Bass kernel for CrossframeGlobalAttentionModule.

Reference computation (N=500000 current vertices, N_PREV=450000 previous,
C=64 channels, G=32 groups):
    h  = h_lv @ W_hidden.T + b_hidden            # [N_PREV, C]
    h  = pad(h, N)                               # zero rows N_PREV..N
    h  = relu(h @ W_conv.T)
    h  = group_norm(h, gamma, beta)              # stats over ALL N rows
    g  = sigmoid((h @ W_conv.T) / (N + C))
    g[N_PREV:] = 1.0
    out = g * lv

Key observations exploited here:
  * Rows >= N_PREV of the padded h are zero, relu keeps them zero, so they
    contribute nothing to the group sums; their gate is overwritten with 1.0.
    Only the first N_PREV rows need the full pipeline; rows N_PREV..N are a
    pure copy of lv.  The zero rows still count toward the group-norm divisor
    (N * C/G elements per group), which is handled analytically.
  * The whole h-pipeline can run in bf16: the pre-sigmoid value is
    ~1e-5 in magnitude, so gate = sigmoid(z) = 0.5 + z/4 + ...; a 0.4%
    relative error in z perturbs the output by ~1e-8 relative.  lv and the
    final gate*lv multiply stay fp32.
  * The group-norm affine (x - mean) * gamma * rstd + beta followed by the
    second W_conv matmul is folded into the matmul:
        Wc @ (s*h + t) = (Wc * s) @ h + Wc @ t
    so phase 2 is a single matmul with runtime-scaled weights plus a
    per-channel bias that rides on the sigmoid activation's bias input.

Distribution: data-parallel over the vertex dim on 8 cores.  Each core gets
56250 rows of h_lv/lv (plus 6250 passthrough rows), stored transposed
([C, rows], packed host-side) so channels sit on SBUF partitions.  Two
28125-row blocks are packed into the 128 partitions and processed with
block-diagonal 128x128 weights, giving full PE/DVE/ACT width.  Group-norm
statistics need one 256-byte AllReduce of per-group sum/sumsq.
"""

import math

import numpy as np
import ml_dtypes

import concourse.bass as bass
import concourse.tile as tile
from concourse import bacc, mybir
from concourse.bass_utils import run_bass_kernel_spmd

# ---- problem constants (hardcoded; kernel.py must be self-contained) ----
N_FULL = 500000
N_PREV = 450000
C = 64
G = 32
EPS = 1e-5
NCORES = 8

RH = N_PREV // NCORES            # 56250 gate rows per core
RP = (N_FULL - N_PREV) // NCORES  # 6250 passthrough rows per core
HALF = RH // 2                   # 28125 packed columns (2 blocks of rows)
CSCALE = 1.0 / (N_FULL + C)
INV_CNT = 1.0 / (N_FULL * (C // G))  # group-norm divisor (zeros included)

FD = 2048    # DMA / DVE chunk width (columns)
FDA = 1024   # ACT / PSUM chunk width
MM = 512     # single-matmul moving-operand width (one PSUM bank, fp32 out)

F32 = mybir.dt.float32
BF16 = mybir.dt.bfloat16
AX = mybir.AxisListType
ALU = mybir.AluOpType
ACTF = mybir.ActivationFunctionType

NT = math.ceil(HALF / FD)    # 14 outer chunks
NA = math.ceil(HALF / FDA)   # 28 ACT chunks


def _ceil_chunks(total, step):
    return [(i, min(step, total - i)) for i in range(0, total, step)]


def build_nc(ncores=NCORES, use_collective=True):
    nc = bacc.Bacc(
        "TRN2", target_bir_lowering=False, debug=False, num_devices=ncores
    )

    hT = nc.dram_tensor("hT", [128, HALF], BF16, kind="ExternalInput").ap()
    lvT = nc.dram_tensor("lvT", [128, HALF], F32, kind="ExternalInput").ap()
    lvTp = nc.dram_tensor("lvTp", [128, RP // 2], F32, kind="ExternalInput").ap()
    whT_d = nc.dram_tensor("whT", [128, 128], BF16, kind="ExternalInput").ap()
    wcT_d = nc.dram_tensor("wcT", [128, 128], BF16, kind="ExternalInput").ap()
    biash_d = nc.dram_tensor("biash", [128, 1], F32, kind="ExternalInput").ap()
    gam_d = nc.dram_tensor("gam", [128, 1], F32, kind="ExternalInput").ap()
    bet_d = nc.dram_tensor("bet", [128, 1], F32, kind="ExternalInput").ap()
    selA_d = nc.dram_tensor("selA", [128, G], F32, kind="ExternalInput").ap()
    selB_d = nc.dram_tensor("selB", [G, 128], F32, kind="ExternalInput").ap()
    # outputs, partition-major: cols 0:HALF gate rows, HALF: passthrough
    outT = nc.dram_tensor(
        "outT", [128, HALF + RP // 2], F32, kind="ExternalOutput"
    ).ap()
    hTv = hT
    lvTv = lvT
    outTv = outT[:, 0:HALF]

    with tile.TileContext(nc) as tc:
        with (
            tc.tile_pool(name="const", bufs=1) as constp,
            tc.tile_pool(name="load", bufs=3) as loadp,
            tc.tile_pool(name="h1p", bufs=2) as h1p,
            tc.tile_pool(name="h2p", bufs=1) as h2p,
            tc.tile_pool(name="gatep", bufs=2) as gatep,
            tc.tile_pool(name="outp", bufs=3) as outp,
            tc.tile_pool(name="statp", bufs=1) as statp,
            tc.tile_pool(name="psA", bufs=2, space="PSUM") as psA,
            tc.tile_pool(name="psB", bufs=2, space="PSUM") as psB,
            tc.tile_pool(name="dram", bufs=1, space="DRAM") as dramp,
        ):
            # ---- constants to SBUF ----
            whT = constp.tile([128, 128], BF16, tag="whT")
            wcT = constp.tile([128, 128], BF16, tag="wcT")
            biash = constp.tile([128, 1], F32, tag="biash")
            gam = constp.tile([128, 1], F32, tag="gam")
            bet = constp.tile([128, 1], F32, tag="bet")
            selA = constp.tile([128, G], F32, tag="selA")
            selB = constp.tile([G, 128], F32, tag="selB")
            nc.sync.dma_start(whT[:], whT_d)
            nc.sync.dma_start(wcT[:], wcT_d)
            nc.sync.dma_start(biash[:], biash_d)
            nc.sync.dma_start(gam[:], gam_d)
            nc.sync.dma_start(bet[:], bet_d)
            nc.sync.dma_start(selA[:], selA_d)
            nc.sync.dma_start(selB[:], selB_d)

            # Dummy Sqrt up front so the ACT sqrt table set loads during the
            # initial DMAs; relu/identity/copy live in every set, so the only
            # mid-kernel table switch left is the post-stats sigmoid load.
            warm = statp.tile([128, 1], F32, tag="warm")
            nc.vector.memset(warm[:], 1.0)
            warm2 = statp.tile([128, 1], F32, tag="warm2")
            nc.scalar.activation(warm2[:], warm[:], ACTF.Sqrt)

            # passthrough rows: single DRAM->DRAM copy (gate == 1.0 there)
            nc.sync.dma_start(outT[:, HALF : HALF + RP // 2], lvTp)

            # h2 stays resident in SBUF for phase 2 (bf16, 56.25KB/partition)
            h2 = h2p.tile([128, HALF], BF16, tag="h2")
            # bn_stats (count, mean, M2) x2 per 512-col sub-chunk
            nsub = len(_ceil_chunks(HALF, MM))
            stat6 = statp.tile([128, 6 * nsub], F32, tag="stat6")

            # ---- phase 1: h2 = relu(Wc_bd @ (Wh_bd @ hT + b)) + stats ----
            for j, (c0, lw) in enumerate(_ceil_chunks(HALF, FD)):
                ht = loadp.tile([128, FD], BF16, tag="ht")
                nc.sync.dma_start(ht[:, 0:lw], hTv[:, c0 : c0 + lw])
                for k, (a0, la) in enumerate(_ceil_chunks(lw, FDA)):
                    pa = psA.tile([128, FDA], F32, tag="a")
                    for m0, lm in _ceil_chunks(la, MM):
                        nc.tensor.matmul(
                            pa[:, m0 : m0 + lm],
                            whT[:],
                            ht[:, a0 + m0 : a0 + m0 + lm],
                            start=True,
                            stop=True,
                        )
                    h1 = h1p.tile([128, FDA], BF16, tag="h1")
                    nc.scalar.activation(
                        h1[:, 0:la], pa[:, 0:la], ACTF.Identity, bias=biash[:, 0:1]
                    )
                    pb = psB.tile([128, FDA], F32, tag="b")
                    for m0, lm in _ceil_chunks(la, MM):
                        nc.tensor.matmul(
                            pb[:, m0 : m0 + lm],
                            wcT[:],
                            h1[:, m0 : m0 + lm],
                            start=True,
                            stop=True,
                        )
                    # relu into resident h2
                    nc.scalar.activation(
                        h2[:, c0 + a0 : c0 + a0 + la],
                        pb[:, 0:la],
                        ACTF.Relu,
                    )
                    # per-512 running stats of the freshly-written h2
                    for s0, ls in _ceil_chunks(la, MM):
                        si = (c0 + a0 + s0) // MM
                        nc.vector.bn_stats(
                            stat6[:, 6 * si : 6 * si + 6],
                            h2[:, c0 + a0 + s0 : c0 + a0 + s0 + ls],
                        )

            # ---- stats: per-partition -> per-group -> AllReduce -> affine ----
            agg = statp.tile([128, 2], F32, tag="agg")
            nc.vector.bn_aggr(agg[:], stat6[:])
            # convert (mean, var) over HALF cols -> (sum, sumsq)
            msq0 = statp.tile([128, 1], F32, tag="msq0")
            nc.vector.tensor_tensor(msq0[:], agg[:, 0:1], agg[:, 0:1], ALU.mult)
            ex20 = statp.tile([128, 1], F32, tag="ex20")
            nc.vector.tensor_tensor(ex20[:], agg[:, 1:2], msq0[:], ALU.add)
            ssum = statp.tile([128, 2], F32, tag="ssum")
            nc.vector.tensor_scalar_mul(ssum[:, 0:1], agg[:, 0:1], float(HALF))
            nc.vector.tensor_scalar_mul(ssum[:, 1:2], ex20[:], float(HALF))
            pg = psA.tile([32, 2], F32, tag="a")
            nc.tensor.matmul(pg[:], selA[:], ssum[:], start=True, stop=True)
            gsb = statp.tile([32, 2], F32, tag="gsb")
            nc.scalar.copy(gsb[:], pg[:])

            cin = dramp.tile([32, 2], F32, tag="cin")
            cout = dramp.tile([32, 2], F32, tag="cout")
            nc.sync.dma_start(cin[:], gsb[:])
            if use_collective:
                nc.gpsimd.collective_compute(
                    "AllReduce",
                    ALU.add,
                    ins=[cin.opt()],
                    outs=[cout.opt()],
                    replica_groups=[list(range(ncores))],
                )
            else:
                nc.sync.dma_start(cout[:], cin[:])
            gall = statp.tile([32, 2], F32, tag="gall")
            nc.sync.dma_start(gall[:], cout[:])

            pbc = psB.tile([128, 2], F32, tag="b")
            nc.tensor.matmul(pbc[:], selB[:], gall[:], start=True, stop=True)
            mean = statp.tile([128, 1], F32, tag="mean")
            ex2 = statp.tile([128, 1], F32, tag="ex2")
            nc.vector.tensor_scalar_mul(mean[:], pbc[:, 0:1], INV_CNT)
            nc.vector.tensor_scalar_mul(ex2[:], pbc[:, 1:2], INV_CNT)
            msq = statp.tile([128, 1], F32, tag="msq")
            nc.vector.tensor_tensor(msq[:], mean[:], mean[:], ALU.mult)
            veps = statp.tile([128, 1], F32, tag="veps")
            nc.vector.tensor_tensor(veps[:], ex2[:], msq[:], ALU.subtract)
            nc.vector.tensor_scalar_add(veps[:], veps[:], EPS)
            urec = statp.tile([128, 1], F32, tag="urec")
            nc.vector.reciprocal(urec[:], veps[:])
            rstd = statp.tile([128, 1], F32, tag="rstd")
            nc.scalar.activation(rstd[:], urec[:], ACTF.Sqrt)

            svec = statp.tile([128, 1], F32, tag="svec")
            nc.vector.tensor_tensor(svec[:], gam[:], rstd[:], ALU.mult)
            mstmp = statp.tile([128, 1], F32, tag="mstmp")
            nc.vector.tensor_tensor(mstmp[:], mean[:], svec[:], ALU.mult)
            tvec = statp.tile([128, 1], F32, tag="tvec")
            nc.vector.tensor_tensor(tvec[:], bet[:], mstmp[:], ALU.subtract)
            tbf = statp.tile([128, 1], BF16, tag="tbf")
            nc.vector.tensor_copy(tbf[:], tvec[:])

            w2 = constp.tile([128, 128], BF16, tag="w2")
            nc.vector.tensor_scalar_mul(w2[:], wcT[:], svec[:, 0:1])
            pbias = psB.tile([128, 1], F32, tag="b")
            nc.tensor.matmul(pbias[:], wcT[:], tbf[:], start=True, stop=True)
            sigb = statp.tile([128, 1], F32, tag="sigb")
            nc.vector.tensor_scalar_mul(sigb[:], pbias[:], CSCALE)

            # ---- phase 2: gate = sigmoid((W2_bd @ h2)*c + sigb); out=gate*lv
            for j, (c0, lw) in enumerate(_ceil_chunks(HALF, FD)):
                lt = loadp.tile([128, FD], F32, tag="lt")
                nc.sync.dma_start(lt[:, 0:lw], lvTv[:, c0 : c0 + lw])
                gate = gatep.tile([128, FD], F32, tag="g")
                for a0, la in _ceil_chunks(lw, FDA):
                    pc = psA.tile([128, FDA], F32, tag="a")
                    for m0, lm in _ceil_chunks(la, MM):
                        nc.tensor.matmul(
                            pc[:, m0 : m0 + lm],
                            w2[:],
                            h2[:, c0 + a0 + m0 : c0 + a0 + m0 + lm],
                            start=True,
                            stop=True,
                        )
                    nc.scalar.activation(
                        gate[:, a0 : a0 + la],
                        pc[:, 0:la],
                        ACTF.Sigmoid,
                        bias=sigb[:, 0:1],
                        scale=CSCALE,
                    )
                ot = outp.tile([128, FD], F32, tag="o")
                nc.vector.tensor_tensor(
                    ot[:, 0:lw], gate[:, 0:lw], lt[:, 0:lw], ALU.mult
                )
                nc.gpsimd.dma_start(outTv[:, c0 : c0 + lw], ot[:, 0:lw])

    nc.compile()
    return nc


_NC_CACHE = None


def _get_nc():
    global _NC_CACHE
    if _NC_CACHE is None:
        _NC_CACHE = build_nc()
    return _NC_CACHE


def _prep_consts(W_hidden, b_hidden, W_conv, gamma, beta):
    whT = np.zeros((128, 128), np.float32)
    wcT = np.zeros((128, 128), np.float32)
    whT[0:64, 0:64] = W_hidden.T
    whT[64:128, 64:128] = W_hidden.T
    wcT[0:64, 0:64] = W_conv.T
    wcT[64:128, 64:128] = W_conv.T
    biash = np.concatenate([b_hidden, b_hidden]).reshape(128, 1).astype(np.float32)
    gam = np.concatenate([gamma, gamma]).reshape(128, 1).astype(np.float32)
    bet = np.concatenate([beta, beta]).reshape(128, 1).astype(np.float32)
    p = np.arange(128)
    selA = ((p[:, None] % 64) // 2 == np.arange(G)[None, :]).astype(np.float32)
    selB = np.ascontiguousarray(selA.T)
    return {
        "whT": whT.astype(ml_dtypes.bfloat16),
        "wcT": wcT.astype(ml_dtypes.bfloat16),
        "biash": biash,
        "gam": gam,
        "bet": bet,
        "selA": selA,
        "selB": selB,
    }


def _pack(x2d):
    """[rows, 64] row-major -> [128, rows//2]: partition b*64+c holds
    channel c of row-block b."""
    rows = x2d.shape[0]
    h = rows // 2
    return np.ascontiguousarray(
        x2d.T.reshape(C, 2, h).swapaxes(0, 1).reshape(128, h)
    )


def _unpack(xp, rows):
    """inverse of _pack: [128, rows//2] -> [rows, 64]"""
    h = rows // 2
    return xp.reshape(2, C, h).swapaxes(0, 1).reshape(C, rows).T


def kernel(lv, h_lv, W_hidden, b_hidden, W_conv, gamma, beta, _trace=False):
    lv = np.asarray(lv, np.float32)
    h_lv = np.asarray(h_lv, np.float32)
    consts = _prep_consts(
        np.asarray(W_hidden, np.float32),
        np.asarray(b_hidden, np.float32),
        np.asarray(W_conv, np.float32),
        np.asarray(gamma, np.float32),
        np.asarray(beta, np.float32),
    )

    in_maps = []
    for i in range(NCORES):
        hs = h_lv[i * RH : (i + 1) * RH]
        ls = lv[i * RH : (i + 1) * RH]
        ps = lv[N_PREV + i * RP : N_PREV + (i + 1) * RP]
        m = dict(consts)
        m["hT"] = _pack(hs).astype(ml_dtypes.bfloat16)
        m["lvT"] = _pack(ls)
        m["lvTp"] = _pack(ps)
        in_maps.append(m)

    nc = _get_nc()
    res = run_bass_kernel_spmd(
        nc, in_maps, core_ids=list(range(NCORES)), trace=_trace
    )

    out = np.empty((N_FULL, C), np.float32)
    for i in range(NCORES):
        o = res.results[i]["outT"]
        out[i * RH : (i + 1) * RH] = _unpack(o[:, 0:HALF], RH)
        out[N_PREV + i * RP : N_PREV + (i + 1) * RP] = _unpack(
            o[:, HALF : HALF + RP // 2], RP
        )
    if _trace:
        return out, res
    return out


# revision 16
# speedup vs baseline: 3.3872x; 1.0186x over previous
"""Trainium2 Bass kernel for CrossframeGlobalAttentionModule.

Reference computation (N=500000 current vertices, N_PREV=450000 previous,
C=64 channels, G=32 groups):
    h  = h_lv @ W_hidden.T + b_hidden            # [N_PREV, C]
    h  = pad(h, N)                               # zero rows N_PREV..N
    h  = relu(h @ W_conv.T)
    h  = group_norm(h, gamma, beta)              # stats over ALL N rows
    g  = sigmoid((h @ W_conv.T) / (N + C))
    g[N_PREV:] = 1.0
    out = g * lv

Key observations exploited here:
  * Rows >= N_PREV of the padded h are zero, relu keeps them zero, so they
    contribute nothing to the group sums; their gate is overwritten with 1.0.
    Only the first N_PREV rows need the full pipeline; rows N_PREV..N are a
    pure copy of lv.  The zero rows still count toward the group-norm divisor
    (N * C/G elements per group), which is handled analytically.
  * The whole h-pipeline can run in bf16: the pre-sigmoid value is
    ~1e-5 in magnitude, so gate = sigmoid(z) = 0.5 + z/4 + ...; a 0.4%
    relative error in z perturbs the output by ~1e-8 relative.  lv and the
    final gate*lv multiply stay fp32.
  * The group-norm affine (x - mean) * gamma * rstd + beta followed by the
    second W_conv matmul is folded into the matmul:
        Wc @ (s*h + t) = (Wc * s) @ h + Wc @ t
    so phase 2 is a single matmul with runtime-scaled weights plus a
    per-channel bias that rides on the sigmoid activation's bias input.

Distribution: data-parallel over the vertex dim on 8 cores.  Each core gets
56250 rows of h_lv/lv (plus 6250 passthrough rows), stored transposed
([C, rows], packed host-side) so channels sit on SBUF partitions.  Two
28125-row blocks are packed into the 128 partitions and processed with
block-diagonal 128x128 weights, giving full PE/DVE/ACT width.  Group-norm
statistics need one 256-byte AllReduce of per-group sum/sumsq.
"""

import math

import numpy as np
import ml_dtypes

import concourse.bass as bass
import concourse.tile as tile
from concourse import bacc, mybir
from concourse.bass_utils import run_bass_kernel_spmd

# ---- problem constants (hardcoded; kernel.py must be self-contained) ----
N_FULL = 500000
N_PREV = 450000
C = 64
G = 32
EPS = 1e-5
NCORES = 8

RH = N_PREV // NCORES            # 56250 gate rows per core
RP = (N_FULL - N_PREV) // NCORES  # 6250 passthrough rows per core
HALF = RH // 2                   # 28125 packed columns (2 blocks of rows)
CSCALE = 1.0 / (N_FULL + C)
INV_CNT = 1.0 / (N_FULL * (C // G))  # group-norm divisor (zeros included)

FD = 2048    # DMA / DVE chunk width (columns)
FDA = 1024   # ACT / PSUM chunk width
MM = 512     # single-matmul moving-operand width (one PSUM bank, fp32 out)

F32 = mybir.dt.float32
BF16 = mybir.dt.bfloat16
AX = mybir.AxisListType
ALU = mybir.AluOpType
ACTF = mybir.ActivationFunctionType

NT = math.ceil(HALF / FD)    # 14 outer chunks
NA = math.ceil(HALF / FDA)   # 28 ACT chunks


def _ceil_chunks(total, step):
    return [(i, min(step, total - i)) for i in range(0, total, step)]


def build_nc(ncores=NCORES, use_collective=True):
    nc = bacc.Bacc(
        "TRN2", target_bir_lowering=False, debug=False, num_devices=ncores
    )

    hT = nc.dram_tensor("hT", [128, HALF], BF16, kind="ExternalInput").ap()
    lvT = nc.dram_tensor("lvT", [128, HALF], F32, kind="ExternalInput").ap()
    lvTp = nc.dram_tensor("lvTp", [128, RP // 2], F32, kind="ExternalInput").ap()
    whT_d = nc.dram_tensor("whT", [128, 128], BF16, kind="ExternalInput").ap()
    wcT_d = nc.dram_tensor("wcT", [128, 128], BF16, kind="ExternalInput").ap()
    biash_d = nc.dram_tensor("biash", [128, 1], F32, kind="ExternalInput").ap()
    gam_d = nc.dram_tensor("gam", [128, 1], F32, kind="ExternalInput").ap()
    bet_d = nc.dram_tensor("bet", [128, 1], F32, kind="ExternalInput").ap()
    selA_d = nc.dram_tensor("selA", [128, G], F32, kind="ExternalInput").ap()
    selB_d = nc.dram_tensor("selB", [G, 128], F32, kind="ExternalInput").ap()
    # outputs, partition-major: cols 0:HALF gate rows, HALF: passthrough
    outT = nc.dram_tensor(
        "outT", [128, HALF + RP // 2], F32, kind="ExternalOutput"
    ).ap()
    hTv = hT
    lvTv = lvT
    outTv = outT[:, 0:HALF]

    with tile.TileContext(nc) as tc:
        with (
            tc.tile_pool(name="const", bufs=1) as constp,
            tc.tile_pool(name="load", bufs=3) as loadp,
            tc.tile_pool(name="lvload", bufs=7) as ltp,
            tc.tile_pool(name="h1p", bufs=3) as h1p,
            tc.tile_pool(name="h2p", bufs=1) as h2p,
            tc.tile_pool(name="gatep", bufs=3) as gatep,
            tc.tile_pool(name="outp", bufs=4) as outp,
            tc.tile_pool(name="statp", bufs=1) as statp,
            tc.tile_pool(name="psA", bufs=2, space="PSUM") as psA,
            tc.tile_pool(name="psB", bufs=2, space="PSUM") as psB,
            tc.tile_pool(name="dram", bufs=1, space="DRAM") as dramp,
        ):
            # ---- constants to SBUF ----
            whT = constp.tile([128, 128], BF16, tag="whT")
            wcT = constp.tile([128, 128], BF16, tag="wcT")
            biash = constp.tile([128, 1], F32, tag="biash")
            gam = constp.tile([128, 1], F32, tag="gam")
            bet = constp.tile([128, 1], F32, tag="bet")
            selA = constp.tile([128, G], F32, tag="selA")
            selB = constp.tile([G, 128], F32, tag="selB")
            nc.sync.dma_start(whT[:], whT_d)
            nc.sync.dma_start(wcT[:], wcT_d)
            nc.sync.dma_start(biash[:], biash_d)
            nc.sync.dma_start(gam[:], gam_d)
            nc.sync.dma_start(bet[:], bet_d)
            nc.sync.dma_start(selA[:], selA_d)
            nc.sync.dma_start(selB[:], selB_d)

            # Dummy Sigmoid up front: loads the sigmoid table set during the
            # initial DMAs.  Identity/relu live in every set and rstd is
            # computed on DVE, so no mid-kernel ACT table switch remains.
            warm = statp.tile([128, 1], F32, tag="warm")
            nc.vector.memset(warm[:], 1.0)
            warm2 = statp.tile([128, 1], F32, tag="warm2")
            nc.scalar.activation(warm2[:], warm[:], ACTF.Sigmoid)

            # passthrough rows: single DRAM->DRAM copy (gate == 1.0 there)
            nc.sync.dma_start(outT[:, HALF : HALF + RP // 2], lvTp)

            # h2 stays resident in SBUF for phase 2 (bf16, 56.25KB/partition)
            h2 = h2p.tile([128, HALF], BF16, tag="h2")
            # bn_stats (count, mean, M2) x2 per 512-col sub-chunk
            nsub = len(_ceil_chunks(HALF, MM))
            stat6 = statp.tile([128, 6 * nsub], F32, tag="stat6")

            # ---- phase 1: h2 = relu(Wc_bd @ (Wh_bd @ hT + b)) + stats ----
            for j, (c0, lw) in enumerate(_ceil_chunks(HALF, FD)):
                ht = loadp.tile([128, FD], BF16, tag="ht")
                nc.sync.dma_start(ht[:, 0:lw], hTv[:, c0 : c0 + lw])
                for k, (a0, la) in enumerate(_ceil_chunks(lw, FDA)):
                    pa = psA.tile([128, FDA], F32, tag="a")
                    for m0, lm in _ceil_chunks(la, MM):
                        nc.tensor.matmul(
                            pa[:, m0 : m0 + lm],
                            whT[:],
                            ht[:, a0 + m0 : a0 + m0 + lm],
                            start=True,
                            stop=True,
                        )
                    h1 = h1p.tile([128, FDA], BF16, tag="h1")
                    nc.scalar.activation(
                        h1[:, 0:la], pa[:, 0:la], ACTF.Identity, bias=biash[:, 0:1]
                    )
                    pb = psB.tile([128, FDA], F32, tag="b")
                    for m0, lm in _ceil_chunks(la, MM):
                        nc.tensor.matmul(
                            pb[:, m0 : m0 + lm],
                            wcT[:],
                            h1[:, m0 : m0 + lm],
                            start=True,
                            stop=True,
                        )
                    # relu into resident h2
                    nc.scalar.activation(
                        h2[:, c0 + a0 : c0 + a0 + la],
                        pb[:, 0:la],
                        ACTF.Relu,
                    )
                    # per-512 running stats of the freshly-written h2
                    for s0, ls in _ceil_chunks(la, MM):
                        si = (c0 + a0 + s0) // MM
                        nc.vector.bn_stats(
                            stat6[:, 6 * si : 6 * si + 6],
                            h2[:, c0 + a0 + s0 : c0 + a0 + s0 + ls],
                        )

            # ---- stats: per-partition -> per-group -> AllReduce -> affine ----
            agg = statp.tile([128, 2], F32, tag="agg")
            nc.vector.bn_aggr(agg[:], stat6[:])
            # convert (mean, var) over HALF cols -> (sum, sumsq)
            msq0 = statp.tile([128, 1], F32, tag="msq0")
            nc.vector.tensor_tensor(msq0[:], agg[:, 0:1], agg[:, 0:1], ALU.mult)
            ex20 = statp.tile([128, 1], F32, tag="ex20")
            nc.vector.tensor_tensor(ex20[:], agg[:, 1:2], msq0[:], ALU.add)
            ssum = statp.tile([128, 2], F32, tag="ssum")
            nc.vector.tensor_scalar_mul(ssum[:, 0:1], agg[:, 0:1], float(HALF))
            nc.vector.tensor_scalar_mul(ssum[:, 1:2], ex20[:], float(HALF))
            pg = psA.tile([32, 2], F32, tag="a")
            nc.tensor.matmul(pg[:], selA[:], ssum[:], start=True, stop=True)
            gsb = statp.tile([32, 2], F32, tag="gsb")
            nc.scalar.copy(gsb[:], pg[:])

            cin = dramp.tile([32, 2], F32, tag="cin")
            cout = dramp.tile([32, 2], F32, tag="cout")
            nc.sync.dma_start(cin[:], gsb[:])
            if use_collective:
                nc.gpsimd.collective_compute(
                    "AllReduce",
                    ALU.add,
                    ins=[cin.opt()],
                    outs=[cout.opt()],
                    replica_groups=[list(range(ncores))],
                )
            else:
                nc.sync.dma_start(cout[:], cin[:])
            gall = statp.tile([32, 2], F32, tag="gall")
            nc.sync.dma_start(gall[:], cout[:])

            pbc = psB.tile([128, 2], F32, tag="b")
            nc.tensor.matmul(pbc[:], selB[:], gall[:], start=True, stop=True)
            mean = statp.tile([128, 1], F32, tag="mean")
            ex2 = statp.tile([128, 1], F32, tag="ex2")
            nc.vector.tensor_scalar_mul(mean[:], pbc[:, 0:1], INV_CNT)
            nc.vector.tensor_scalar_mul(ex2[:], pbc[:, 1:2], INV_CNT)
            msq = statp.tile([128, 1], F32, tag="msq")
            nc.vector.tensor_tensor(msq[:], mean[:], mean[:], ALU.mult)
            veps = statp.tile([128, 1], F32, tag="veps")
            nc.vector.tensor_tensor(veps[:], ex2[:], msq[:], ALU.subtract)
            nc.vector.tensor_scalar_add(veps[:], veps[:], EPS)
            # rstd = rsqrt(var+eps) via Newton on DVE; var+eps is O(0.2..0.5)
            # here so y0=2 converges quadratically (4 iters ~ fp32 exact)
            rstd = statp.tile([128, 1], F32, tag="rstd")
            nc.vector.memset(rstd[:], 2.0)
            nt1 = statp.tile([128, 1], F32, tag="nt1")
            nt2 = statp.tile([128, 1], F32, tag="nt2")
            for _ in range(4):
                nc.vector.tensor_tensor(nt1[:], veps[:], rstd[:], ALU.mult)
                nc.vector.tensor_tensor(nt2[:], nt1[:], rstd[:], ALU.mult)
                nc.vector.tensor_scalar(
                    nt1[:], nt2[:], -0.5, 1.5, ALU.mult, ALU.add
                )
                nc.vector.tensor_tensor(rstd[:], rstd[:], nt1[:], ALU.mult)

            svec = statp.tile([128, 1], F32, tag="svec")
            nc.vector.tensor_tensor(svec[:], gam[:], rstd[:], ALU.mult)
            mstmp = statp.tile([128, 1], F32, tag="mstmp")
            nc.vector.tensor_tensor(mstmp[:], mean[:], svec[:], ALU.mult)
            tvec = statp.tile([128, 1], F32, tag="tvec")
            nc.vector.tensor_tensor(tvec[:], bet[:], mstmp[:], ALU.subtract)
            tbf = statp.tile([128, 1], BF16, tag="tbf")
            nc.vector.tensor_copy(tbf[:], tvec[:])

            w2 = constp.tile([128, 128], BF16, tag="w2")
            nc.vector.tensor_scalar_mul(w2[:], wcT[:], svec[:, 0:1])
            pbias = psB.tile([128, 1], F32, tag="b")
            nc.tensor.matmul(pbias[:], wcT[:], tbf[:], start=True, stop=True)
            sigb = statp.tile([128, 1], F32, tag="sigb")
            nc.vector.tensor_scalar_mul(sigb[:], pbias[:], CSCALE)

            # ---- phase 2: gate = sigmoid((W2_bd @ h2)*c + sigb); out=gate*lv
            # lv tiles have no deps on phase 1 / stats, so with a deep pool
            # they prefetch during the collective window.
            lts = {}
            for j, (c0, lw) in enumerate(_ceil_chunks(HALF, FD)):
                lt = ltp.tile([128, FD], F32, tag="lt")
                nc.sync.dma_start(lt[:, 0:lw], lvTv[:, c0 : c0 + lw])
                lts[c0] = lt
            for idx, (a0, la) in enumerate(_ceil_chunks(HALF, FDA)):
                pcp = psA if idx % 2 == 0 else psB
                pc = pcp.tile([128, FDA], F32, tag="a" if idx % 2 == 0 else "b")
                for m0, lm in _ceil_chunks(la, MM):
                    nc.tensor.matmul(
                        pc[:, m0 : m0 + lm],
                        w2[:],
                        h2[:, a0 + m0 : a0 + m0 + lm],
                        start=True,
                        stop=True,
                    )
                gate = gatep.tile([128, FDA], F32, tag="g")
                nc.scalar.activation(
                    gate[:, 0:la],
                    pc[:, 0:la],
                    ACTF.Sigmoid,
                    bias=sigb[:, 0:1],
                    scale=CSCALE,
                )
                c0 = (a0 // FD) * FD
                lt = lts[c0]
                ot = outp.tile([128, FDA], F32, tag="o")
                nc.vector.tensor_tensor(
                    ot[:, 0:la], gate[:, 0:la], lt[:, a0 - c0 : a0 - c0 + la],
                    ALU.mult,
                )
                nc.gpsimd.dma_start(outTv[:, a0 : a0 + la], ot[:, 0:la])

    nc.compile()
    return nc


_NC_CACHE = None


def _get_nc():
    global _NC_CACHE
    if _NC_CACHE is None:
        _NC_CACHE = build_nc()
    return _NC_CACHE


def _prep_consts(W_hidden, b_hidden, W_conv, gamma, beta):
    whT = np.zeros((128, 128), np.float32)
    wcT = np.zeros((128, 128), np.float32)
    whT[0:64, 0:64] = W_hidden.T
    whT[64:128, 64:128] = W_hidden.T
    wcT[0:64, 0:64] = W_conv.T
    wcT[64:128, 64:128] = W_conv.T
    biash = np.concatenate([b_hidden, b_hidden]).reshape(128, 1).astype(np.float32)
    gam = np.concatenate([gamma, gamma]).reshape(128, 1).astype(np.float32)
    bet = np.concatenate([beta, beta]).reshape(128, 1).astype(np.float32)
    p = np.arange(128)
    selA = ((p[:, None] % 64) // 2 == np.arange(G)[None, :]).astype(np.float32)
    selB = np.ascontiguousarray(selA.T)
    return {
        "whT": whT.astype(ml_dtypes.bfloat16),
        "wcT": wcT.astype(ml_dtypes.bfloat16),
        "biash": biash,
        "gam": gam,
        "bet": bet,
        "selA": selA,
        "selB": selB,
    }


def _pack(x2d):
    """[rows, 64] row-major -> [128, rows//2]: partition b*64+c holds
    channel c of row-block b."""
    rows = x2d.shape[0]
    h = rows // 2
    return np.ascontiguousarray(
        x2d.T.reshape(C, 2, h).swapaxes(0, 1).reshape(128, h)
    )


def _unpack(xp, rows):
    """inverse of _pack: [128, rows//2] -> [rows, 64]"""
    h = rows // 2
    return xp.reshape(2, C, h).swapaxes(0, 1).reshape(C, rows).T


def kernel(lv, h_lv, W_hidden, b_hidden, W_conv, gamma, beta, _trace=False):
    lv = np.asarray(lv, np.float32)
    h_lv = np.asarray(h_lv, np.float32)
    consts = _prep_consts(
        np.asarray(W_hidden, np.float32),
        np.asarray(b_hidden, np.float32),
        np.asarray(W_conv, np.float32),
        np.asarray(gamma, np.float32),
        np.asarray(beta, np.float32),
    )

    in_maps = []
    for i in range(NCORES):
        hs = h_lv[i * RH : (i + 1) * RH]
        ls = lv[i * RH : (i + 1) * RH]
        ps = lv[N_PREV + i * RP : N_PREV + (i + 1) * RP]
        m = dict(consts)
        m["hT"] = _pack(hs).astype(ml_dtypes.bfloat16)
        m["lvT"] = _pack(ls)
        m["lvTp"] = _pack(ps)
        in_maps.append(m)

    nc = _get_nc()
    res = run_bass_kernel_spmd(
        nc, in_maps, core_ids=list(range(NCORES)), trace=_trace
    )

    out = np.empty((N_FULL, C), np.float32)
    for i in range(NCORES):
        o = res.results[i]["outT"]
        out[i * RH : (i + 1) * RH] = _unpack(o[:, 0:HALF], RH)
        out[N_PREV + i * RP : N_PREV + (i + 1) * RP] = _unpack(
            o[:, HALF : HALF + RP // 2], RP
        )
    if _trace:
        return out, res
    return out
